# revision 61
# baseline (speedup 1.0000x reference)
"""ABC-Conv (binary conv with multiple estimators) on 8 trn2 NeuronCores.

Math: reference computes
    xq   = sign(x)
    beta = boxfilter3x3(sum_c |x|) / (3*3*128)            [B,110,110]
    out  = sum_e conv(xq, sign(kernels[e])) * beta[...,None] * alphas[e]

conv is linear in its kernel and alphas[e] scales output channels, so the
estimator loop folds into ONE conv with W = sum_e sign(kernels[e]) * alphas[e]:
    out = beta[..., None] * conv(xq, W)

Sharding: data-parallel over batch, 2 images per core, weights replicated.

Per-core kernel:
  - x arrives as [25088, 128] f32 (2 images, flat pixel-major, c contiguous)
  - sign+cast to bf16 on ScalarE; channel |x| sums on VectorE (for beta)
  - xqT [cin, flatpix]: image 0 is transposed on the TensorEngine (idle during
    the input phase; lowest latency), image 1 through a DRAM bounce + x-bar
    DMA transpose (runs on otherwise-idle DMA capacity during image 0's conv)
  - conv = per 128-pixel tile: 9 accumulated bf16 matmuls (shifted flat slices)
  - beta box filter = 3 tiny matmuls against host-built 0/1 shift matrices
  - PSUM -> SBUF copy applies beta as a per-partition scale; output staged
    bf16 and stored 8 tiles (0.5 MiB) per DMA; host casts back to f32
  - emission is interleaved chunk-by-chunk so conv groups unblock as soon as
    their input coverage exists (Tile priority follows emission order)

The flat-pixel trick: out[p] = sum_{kh,kw} xq[p + kh*112 + kw] for flat
p = h*112 + w.  Columns w in {110,111} are garbage (wrap into next row) and
get sliced away on the host; 2 zero-pad rows per image terminate the bottom.
"""

import sys

sys.path.insert(0, "/opt/trn_rl_repo")

import ml_dtypes
import numpy as np

import concourse.tile as tile
from concourse import bacc, mybir
from concourse.bass import ds
from concourse.bass_utils import run_bass_kernel_spmd

F32 = mybir.dt.float32
BF16 = mybir.dt.bfloat16

N_CORES = 8
B_PER_CORE = 2
H = W_IMG = 112
CIN = 128
F = 256
E = 3
D_DIM = 9 * CIN  # 1152

IMG_PIX = H * W_IMG  # 12544 = 98 * 128
IMG_PAD = (H + 2) * W_IMG  # 12768 (2 zero rows terminate the window reads)
OUT_ROWS = 110
OUT_PIX = OUT_ROWS * W_IMG  # 12320 = 96*128 + 32
N_OUT_TILES = 97  # 96 full tiles + one 32-row tile
S_SEG = 100  # per-image column segment in the |x|-sum buffer (98 real + 2 pad)
DOFF = [kh * W_IMG + kw for kh in range(3) for kw in range(3)]

CHUNK = 2048  # pixels per load/sign/transpose chunk (16 tiles)
CHUNKS = [(i * CHUNK, CHUNK) for i in range(6)] + [(6 * CHUNK, IMG_PIX - 6 * CHUNK)]
# image 0 starts with smaller chunks so its conv can begin sooner
CHUNKS0 = [(0, 1024), (1024, 1024)] + CHUNKS[1:]
OGROUP = 8  # out tiles per store DMA


def _box_matrices():
    """beta_pre[p, t] = sum_q sum_k Mq[k,p] * s[k, t+q]
    where s[:, t] holds channel-abs-sums of flat pixels t*128..t*128+127.
    Window offsets reach p+353, spanning three 128-columns of s."""
    ms = np.zeros((3, 128, 128), np.float32)
    for p in range(128):
        for d in DOFF:
            k = p + d
            ms[k // 128, k % 128, p] = 1.0
    return ms.astype(ml_dtypes.bfloat16)


def build_nc():
    nc = bacc.Bacc("TRN2", target_bir_lowering=False, debug=False)
    x_d = nc.dram_tensor("x", [B_PER_CORE * IMG_PIX, CIN], F32, kind="ExternalInput").ap()
    k_d = nc.dram_tensor("kernels", [E * 9, CIN, F], BF16, kind="ExternalInput").ap()
    a_d = nc.dram_tensor("alphas", [1, E * 9 * F], BF16, kind="ExternalInput").ap()
    m_d = nc.dram_tensor("boxm", [3, 128, 128], BF16, kind="ExternalInput").ap()
    i_d = nc.dram_tensor("ident", [128, 128], BF16, kind="ExternalInput").ap()
    o_d = nc.dram_tensor("out", [B_PER_CORE * OUT_PIX, F], BF16, kind="ExternalOutput").ap()
    xq_d = nc.dram_tensor("xq_scratch", [IMG_PIX, CIN], BF16).ap()  # image 1 bounce

    with tile.TileContext(nc) as tc:
        with (
            tc.tile_pool(name="const", bufs=1) as constp,
            tc.tile_pool(name="xin", bufs=3) as xinp,
            tc.tile_pool(name="xq", bufs=3) as xqp,
            tc.tile_pool(name="big", bufs=1) as bigp,
            tc.tile_pool(name="outs", bufs=3) as outp,
            tc.tile_pool(name="psum", bufs=6, space="PSUM") as psump,
            tc.tile_pool(name="psb", bufs=1, space="PSUM") as psbp,
            tc.tile_pool(name="ptr", bufs=1, space="PSUM") as ptrp,
        ):
            # ---------- constants & weight fold ----------
            boxm_bf = constp.tile([128, 3, 128], BF16)
            nc.gpsimd.dma_start(boxm_bf[:, :, :], m_d.rearrange("m k p -> k m p"))
            ident = constp.tile([128, 128], BF16)
            nc.gpsimd.dma_start(ident[:, :], i_d[:, :])
            w_bf = constp.tile([128, 9 * F], BF16)
            with tc.tile_pool(name="fold", bufs=1) as foldp:
                alpha_row = foldp.tile([1, E * 9 * F], BF16, tag="arow")
                nc.gpsimd.dma_start(alpha_row[:, :], a_d[:, :])
                wm = []
                for e in range(E):
                    kst = foldp.tile([128, 9, F], BF16, tag="kst", bufs=2)
                    nc.sync.dma_start(
                        kst[:, :, :],
                        k_d[e * 9 : (e + 1) * 9, :, :].rearrange("j c f -> c j f"),
                    )
                    abc = foldp.tile([128, 9 * F], BF16, tag="abc", bufs=2)
                    nc.gpsimd.partition_broadcast(
                        abc[:, :], alpha_row[:, ds(e * 9 * F, 9 * F)]
                    )
                    ksgn = foldp.tile([128, 9 * F], BF16, tag="ksgn", bufs=2)
                    nc.scalar.sign(ksgn[:, :], kst[:, :, :].rearrange("c j f -> c (j f)"))
                    km = foldp.tile([128, 9 * F], BF16, tag=f"km{e}")
                    nc.vector.tensor_mul(km[:, :], ksgn[:, :], abc[:, :])
                    wm.append(km)
                w01 = foldp.tile([128, 9 * F], BF16, tag="w01")
                nc.vector.tensor_add(w01[:, :], wm[0][:, :], wm[1][:, :])
                nc.vector.tensor_add(w_bf[:, :], w01[:, :], wm[2][:, :])

            # ---------- big persistent buffers ----------
            xqT = bigp.tile([128, B_PER_CORE * IMG_PAD], BF16)  # [cin, flat pix]
            s_f = bigp.tile([128, B_PER_CORE * S_SEG], F32)  # channel |x| sums
            s_bf = bigp.tile([128, B_PER_CORE * S_SEG], BF16)
            beta = bigp.tile([128, B_PER_CORE * N_OUT_TILES], F32)
            for b in range(B_PER_CORE):
                nc.vector.memset(xqT[:, ds(b * IMG_PAD + IMG_PIX, IMG_PAD - IMG_PIX)], 0.0)
                nc.vector.memset(s_bf[:, ds(b * S_SEG + 98, 2)], 0.0)

            ncopy = 0  # alternates the PSUM->SBUF copy engine

            def emit_beta(b, c0, cn):
                bps = psbp.tile([128, 32], F32, tag="bps")
                for q in range(3):
                    nc.tensor.matmul(
                        bps[:, :cn],
                        lhsT=boxm_bf[:, q, :],
                        rhs=s_bf[:, ds(b * S_SEG + c0 + q, cn)],
                        start=(q == 0),
                        stop=(q == 2),
                    )
                nc.vector.tensor_scalar_mul(
                    beta[:, ds(b * N_OUT_TILES + c0, cn)], bps[:, :cn], 1.0 / D_DIM
                )

            def emit_conv_group(b, t, gsize):
                nonlocal ncopy
                ostage = outp.tile([128, OGROUP, F], BF16, tag="ostage")
                for k in range(gsize):
                    ps = psump.tile([128, F], F32, tag="ps")
                    base = b * IMG_PAD + (t + k) * 128
                    for i, dlt in enumerate(DOFF):
                        nc.tensor.matmul(
                            ps[:, :],
                            lhsT=xqT[:, ds(base + dlt, 128)],
                            rhs=w_bf[:, ds(i * F, F)],
                            start=(i == 0),
                            stop=(i == 8),
                        )
                    scale_ap = beta[:, ds(b * N_OUT_TILES + t + k, 1)]
                    if ncopy % 2 == 0:
                        nc.vector.tensor_scalar_mul(ostage[:, k, :], ps[:, :], scale_ap)
                    else:
                        nc.scalar.activation(
                            ostage[:, k, :], ps[:, :],
                            mybir.ActivationFunctionType.Copy, scale=scale_ap,
                        )
                    ncopy += 1
                rows = min(128 * gsize, OUT_PIX - t * 128)
                r0 = b * OUT_PIX + t * 128
                if rows >= 128:
                    nc.gpsimd.dma_start(
                        o_d[r0 : r0 + rows, :].rearrange("(j p) f -> p j f", p=128),
                        ostage[:, : rows // 128, :],
                    )
                else:
                    nc.gpsimd.dma_start(o_d[r0 : r0 + rows, :], ostage[:rows, 0, :])

            def emit_chunk(b, c0, npix):
                nt = npix // 128
                row0 = b * IMG_PIX + c0
                xst = xinp.tile([128, CHUNK // 128, CIN], F32, tag="xst")
                nc.sync.dma_start(
                    xst[:, :nt, :],
                    x_d[row0 : row0 + npix, :].rearrange("(j p) c -> p j c", p=128),
                )
                xqst = xqp.tile([128, CHUNK // 128, CIN], BF16, tag="xqst")
                nc.scalar.sign(xqst[:, :nt, :], xst[:, :nt, :])
                if b == 0:
                    # TensorE transpose path (PE idle during input phase)
                    for j in range(0, nt, 2):
                        ptr = ptrp.tile([128, 2, 128], BF16, tag="ptr")
                        for jj in range(2):
                            nc.tensor.transpose(
                                ptr[:, jj, :], xqst[:, j + jj, :], ident[:, :]
                            )
                        dst = xqT[:, ds(b * IMG_PAD + c0 + j * 128, 256)].rearrange(
                            "p (a c) -> p a c", a=2
                        )
                        if (c0 // 128 + j) % 4 == 0:
                            nc.vector.tensor_copy(dst, ptr[:, :, :])
                        else:
                            nc.scalar.copy(dst, ptr[:, :, :])
                else:
                    # DRAM bounce + x-bar DMA transpose path; issued on Sync,
                    # which is idle once loads drain and carries no ops that
                    # depend on conv progress (no head-of-line blocking)
                    nc.sync.dma_start(
                        xq_d[c0 : c0 + npix, :].rearrange("(j p) c -> p j c", p=128),
                        xqst[:, :nt, :],
                    )
                    nc.sync.dma_start(
                        xqT[:, ds(b * IMG_PAD + c0, npix)],
                        xq_d[c0 : c0 + npix, :],
                        transpose=True,
                    )
                nc.vector.tensor_reduce(
                    s_f[:, ds(b * S_SEG + c0 // 128, nt)],
                    xst[:, :nt, :],
                    axis=mybir.AxisListType.X,
                    op=mybir.AluOpType.add,
                    apply_absolute_value=True,
                )
                nc.vector.tensor_copy(
                    s_bf[:, ds(b * S_SEG + c0 // 128, nt)],
                    s_f[:, ds(b * S_SEG + c0 // 128, nt)],
                )

            BCH = ((0, 9), (9, 16), (25, 25), (50, 25), (75, 22))
            for b in range(B_PER_CORE):
                bq = 0
                bcov = 0
                gt = 0
                chunks = CHUNKS0 if b == 0 else CHUNKS
                for c, (c0, npix) in enumerate(chunks):
                    last = c == len(chunks) - 1
                    emit_chunk(b, c0, npix)
                    scols = 100 if last else (c0 + npix) // 128  # s_bf cols ready
                    pcov = IMG_PAD if last else c0 + npix  # xqT cols ready
                    while bq < len(BCH) and BCH[bq][0] + BCH[bq][1] + 2 <= scols:
                        emit_beta(b, *BCH[bq])
                        bcov = BCH[bq][0] + BCH[bq][1]
                        bq += 1
                    while gt < N_OUT_TILES:
                        gs = min(
                            OGROUP, N_OUT_TILES - gt, bcov - gt, (pcov - 226) // 128 - gt
                        )
                        if gt < 96:  # keep the partial last tile in its own group
                            gs = min(gs, 96 - gt)
                        if gs <= 0:
                            break
                        emit_conv_group(b, gt, gs)
                        gt += gs

    nc.compile()
    return nc


_NC = None


def _get_nc():
    global _NC
    if _NC is None:
        _NC = build_nc()
    return _NC


def _in_maps(x, kernels, alphas):
    x = np.ascontiguousarray(np.asarray(x, np.float32))
    kernels = np.asarray(kernels, np.float32)
    alphas = np.asarray(alphas, np.float32)
    # bf16 round-to-nearest preserves the sign of every weight, and only
    # sign(kernels) enters the math -- so the kernels can ship as bf16
    kf = np.ascontiguousarray(kernels.reshape(E * 9, CIN, F).astype(ml_dtypes.bfloat16))
    # pre-tile alphas across taps (bf16); broadcast across partitions on-device
    af = np.ascontiguousarray(
        np.tile(alphas[:, None, :], (1, 9, 1)).reshape(1, E * 9 * F).astype(ml_dtypes.bfloat16)
    )
    boxm = _box_matrices()
    ident = np.eye(128, dtype=ml_dtypes.bfloat16)
    xs = x.reshape(N_CORES, B_PER_CORE * IMG_PIX, CIN)
    return [
        {
            "x": np.ascontiguousarray(xs[c]),
            "kernels": kf,
            "alphas": af,
            "boxm": boxm,
            "ident": ident,
        }
        for c in range(N_CORES)
    ]


def _gather(results):
    outs = []
    for c in range(N_CORES):
        o = np.asarray(results[c]["out"]).reshape(B_PER_CORE, OUT_ROWS, W_IMG, F)
        outs.append(o[:, :, :OUT_ROWS, :].astype(np.float32))
    return np.ascontiguousarray(np.concatenate(outs, axis=0))


def kernel(x, kernels, alphas):
    nc = _get_nc()
    res = run_bass_kernel_spmd(nc, _in_maps(x, kernels, alphas), core_ids=list(range(N_CORES)))
    return _gather(res.results)


def _install_profile_hook():
    """The agent image's antenv lacks axon_hooks; recreate it so
    run_bass_kernel_spmd(trace=True) can NTFF-profile via libaxon_pjrt.so."""
    import types

    import antenv

    if "antenv.axon_hooks" in sys.modules:
        return
    mod = types.ModuleType("antenv.axon_hooks")
    holder = {}
    mod.set_axon_ntff_profile_hook = lambda h: holder.__setitem__("h", h)
    mod.get_axon_ntff_profile_hook = lambda: holder.get("h")
    sys.modules["antenv.axon_hooks"] = mod
    antenv.axon_hooks = mod

    from trn_agent_boot.trn_boot import _ntff_profile_via_ctypes

    hook = _ntff_profile_via_ctypes("/opt/axon/libaxon_pjrt.so")
    mod.set_axon_ntff_profile_hook(hook)

    # upload_artifacts wants a cloud bucket; keep everything local instead.
    import concourse.bass_utils as bu

    bu.upload_artifacts = lambda tmpdir: tmpdir


def run_profiled(x, kernels, alphas, tmpdir=None):
    """Returns (output, exec_time_ns, profile_json_path)."""
    _install_profile_hook()
    nc = _get_nc()
    res = run_bass_kernel_spmd(
        nc,
        _in_maps(x, kernels, alphas),
        core_ids=list(range(N_CORES)),
        trace=True,
        tmpdir=tmpdir,
    )
    return _gather(res.results), res.exec_time_ns, res.profile_json
